# revision 1
# baseline (speedup 1.0000x reference)
"""Trainium2 Bass kernel for nn_MultiHeadAttention_72877005078903.

MHA with ALiBi + causal mask: B=2, T=2048, D=1024, 16 heads, head_dim 64.

Sharding (8 cores): batch x head-quartile. Core c in {0..3} handles batch 0,
cores 4..7 batch 1. Within a batch group, core c owns heads {c, c+4, c+8, c+12}
(one head per ALiBi-slope quartile) so every core's 4 head-slots share the same
per-slot causal/ALiBi block window, keeping the SPMD program identical across
cores while balancing work.

Device-side structure:
- Scores are computed transposed (S.T[j,i]) so the exp output in SBUF is
  directly the lhsT of the attention*V matmul (no transposes in the hot loop).
- ALiBi bias slope*(j-i) is fused into the score matmul as two extra fp32
  contraction rows: lhsT=[kT; slope*j; 1], rhs=[qT; 1; -slope*i].
- A ones-column appended to V makes the softmax denominator fall out of the
  same PSUM accumulation (no max-subtraction needed: scores are O(1) and the
  ALiBi bias is <= 0 on the causal side).
- Per-slot block window: blocks with slope*(distance) >= 24 contribute < 2e-7
  of the row mass and are skipped.
- Causal mask on the diagonal block is applied additively (-1e30) on the PSUM
  scores before exp, avoiding inf*0 NaNs.
- The context is transposed per 128-block on the PE (identity matmul) and fed
  straight into the output projection; partials are summed on the host, where
  bp + bv @ Wp.T is also folded in.
"""

import numpy as np
import ml_dtypes

B, T, D, NH, HD = 2, 2048, 1024, 16, 64
HPC = 4                      # head-slots per core
NB = T // 128                # 16 row blocks
NKK = D // 128               # 8 contraction tiles for projections
NTC = 4                      # 512-wide t-chunks
D_KEEP = (2, 4, 13, 16)      # per-slot kept block-diagonal width
SCALE = float(D) ** 0.25
BF16 = ml_dtypes.bfloat16
NEG = -1.0e30

_PROG = None


def _build_program(nrep=1, fuse_out=True):
    """nrep>1 repeats the whole kernel body back-to-back in one NEFF —
    used only to measure device time as a wall-clock delta."""
    import concourse.bacc as bacc
    import concourse.tile as tile
    from concourse import mybir

    f32 = mybir.dt.float32
    bf16 = mybir.dt.bfloat16
    Exp = mybir.ActivationFunctionType.Exp
    Copy = mybir.ActivationFunctionType.Copy

    nc = bacc.Bacc()
    xT_d = nc.declare_dram_parameter("xT", [D, T], bf16, isOutput=False)
    wq_d = nc.declare_dram_parameter("wq", [D, 256], bf16, isOutput=False)
    wk_d = nc.declare_dram_parameter("wk", [D, 256], bf16, isOutput=False)
    wv_d = nc.declare_dram_parameter("wv", [D, 256], bf16, isOutput=False)
    wp_d = nc.declare_dram_parameter("wp", [256, D], bf16, isOutput=False)
    bq_d = nc.declare_dram_parameter("bq", [128, HPC], f32, isOutput=False)
    bk_d = nc.declare_dram_parameter("bk", [128, HPC], f32, isOutput=False)
    qr_d = nc.declare_dram_parameter("qrows", [HPC, 2, T], f32, isOutput=False)
    kr_d = nc.declare_dram_parameter("krows", [HPC, 2, T], f32, isOutput=False)
    mk_d = nc.declare_dram_parameter("maskadd", [128, 128], bf16, isOutput=False)
    id_d = nc.declare_dram_parameter("ident", [128, 128], bf16, isOutput=False)
    out_d = nc.declare_dram_parameter("out", [T, D], f32, isOutput=True)

    from contextlib import ExitStack

    with tile.TileContext(nc) as tc:
        with ExitStack() as ctx:
            consts = ctx.enter_context(tc.tile_pool(name="consts", bufs=1))
            qkp = ctx.enter_context(tc.tile_pool(name="qkp", bufs=1))
            vp = ctx.enter_context(tc.tile_pool(name="vp", bufs=1))
            ptp = ctx.enter_context(tc.tile_pool(name="ptp", bufs=20))
            cxp = ctx.enter_context(tc.tile_pool(name="cxp", bufs=1))
            small = ctx.enter_context(tc.tile_pool(name="small", bufs=8))
            ps_m = ctx.enter_context(tc.tile_pool(name="ps_m", bufs=2, space="PSUM"))

            def emit_once():
                # ---- inputs to SBUF ----
                # wq first (first matmul needs it), then xT t-chunk-major so the
                # first projection pass streams at DMA arrival rate.
                wq_s = consts.tile([128, NKK, 256], bf16, name="wq_s", tag="wq_s")
                nc.sync.dma_start(wq_s[:], wq_d.rearrange("(o p) m -> p o m", p=128))
                xs = [[None] * NTC for _ in range(NKK)]
                for tcq in range(NTC):
                    for kk in range(NKK):
                        t = consts.tile([128, 512], bf16, name=f"x{kk}_{tcq}",
                                        tag=f"x{kk}_{tcq}")
                        nc.sync.dma_start(
                            t[:], xT_d[kk * 128:(kk + 1) * 128,
                                       512 * tcq:512 * (tcq + 1)])
                        xs[kk][tcq] = t
                # other weights on the scalar-engine HWDGE queue (parallel
                # to sync; gpsimd SWDGE has ~1us per-DMA overhead)
                wk_s = consts.tile([128, NKK, 256], bf16, name="wk_s", tag="wk_s")
                nc.scalar.dma_start(wk_s[:], wk_d.rearrange("(o p) m -> p o m", p=128))
                wv_s = consts.tile([128, NKK, 256], bf16, name="wv_s", tag="wv_s")
                nc.scalar.dma_start(wv_s[:], wv_d.rearrange("(o p) m -> p o m", p=128))
                wp_s = consts.tile([128, 2, D], bf16, name="wp_s", tag="wp_s")
                nc.scalar.dma_start(wp_s[:], wp_d.rearrange("(o p) m -> p o m", p=128))
                bq_s = consts.tile([128, HPC], f32, name="bq_s", tag="bq_s")
                nc.scalar.dma_start(bq_s[:], bq_d[:])
                bk_s = consts.tile([128, HPC], f32, name="bk_s", tag="bk_s")
                nc.scalar.dma_start(bk_s[:], bk_d[:])
                mcl_s = consts.tile([128, 128], bf16, name="mcl_s", tag="mcl_s")
                nc.scalar.dma_start(mcl_s[:], mk_d[:])
                id_s = consts.tile([128, 128], bf16, name="id_s", tag="id_s")
                nc.scalar.dma_start(id_s[:], id_d[:])
                zbias = consts.tile([128, 1], f32, name="zbias", tag="zbias")
                nc.vector.memset(zbias[:], 0.0)

                # Head-slot q/k layout (fp32, [128, T] tiles):
                #  even slot: rows 0:64 = qT/kT data, rows 64:66 = alibi rows;
                #             score matmuls contract over partitions [0:66].
                #  odd slot:  rows 0:62 zeroed, rows 62:64 = alibi rows,
                #             rows 64:128 = qT/kT data (same lanes as the PSUM
                #             half it's copied from); contract over [0:128]
                #             (SBUF APs >32 partitions must start at 0 or 64).
                q_att, k_att = [], []
                for s in range(HPC):
                    qa = qkp.tile([128, T], f32, name=f"qa{s}", tag=f"qa{s}")
                    ka = qkp.tile([128, T], f32, name=f"ka{s}", tag=f"ka{s}")
                    if s % 2 == 0:
                        nc.scalar.dma_start(qa[64:66, :], qr_d[s])
                        nc.scalar.dma_start(ka[64:66, :], kr_d[s])
                    else:
                        nc.gpsimd.memset(qa[0:64, :], 0.0)
                        nc.gpsimd.memset(ka[0:64, :], 0.0)
                        nc.scalar.dma_start(qa[62:64, :], qr_d[s])
                        nc.scalar.dma_start(ka[62:64, :], kr_d[s])
                    q_att.append(qa)
                    k_att.append(ka)
                v4 = vp.tile([128, NB, HPC, 65], bf16, name="v4", tag="v4")
                nc.gpsimd.memset(v4[:, :, :, 64:65], 1.0)
                ctx_sb = cxp.tile([128, NB, 256], bf16, name="ctx_sb", tag="ctx_sb")

                # ---- projections ----
                def proj_qk_tc(w_s, b_s, att, m, tcq):
                    tsl = slice(512 * tcq, 512 * (tcq + 1))
                    ps = ps_m.tile([128, 512], f32, name="psmm", tag="psmm")
                    for kk in range(NKK):
                        nc.tensor.matmul(
                            ps[:],
                            w_s[:, kk, 128 * m:128 * (m + 1)],
                            xs[kk][tcq][:],
                            start=(kk == 0),
                            stop=(kk == NKK - 1),
                        )
                    s_ev, s_od = 2 * m, 2 * m + 1
                    nc.vector.tensor_scalar_add(
                        att[s_ev][0:64, tsl], ps[0:64, :],
                        b_s[0:64, s_ev:s_ev + 1])
                    nc.vector.tensor_scalar_add(
                        att[s_od][64:128, tsl], ps[64:128, :],
                        b_s[64:128, s_od:s_od + 1])

                def proj_v():
                    for tb in range(NB):
                        ps = ps_m.tile([128, 512], f32, name="psmm", tag="psmm")
                        for kk in range(NKK):
                            nc.tensor.matmul(
                                ps[:, 0:256],
                                xs[kk][tb // 4][:, 128 * (tb % 4):128 * (tb % 4 + 1)],
                                wv_s[:, kk, :],
                                start=(kk == 0),
                                stop=(kk == NKK - 1),
                            )
                        nc.vector.tensor_copy(
                            v4[:, tb, :, 0:64],
                            ps[:, 0:256].rearrange("p (s d) -> p s d", d=64))

                # ---- attention for one head-slot ----
                def attention(s, ps_s, ps_c, fuse=None):
                    d = D_KEEP[s]
                    kr = slice(0, 66) if s % 2 == 0 else slice(0, 128)
                    for IC in range(2):
                        ic_lo, ic_hi = 1024 * IC, 1024 * (IC + 1)
                        # Collect each j0's kept i-window, then bin-pack the
                        # regions into shared [128,1024] PSUM tiles so one exp
                        # instruction covers several regions (the scalar
                        # engine's per-instruction overhead dominates small
                        # activations).
                        regions = []
                        for j0 in range(NB):
                            lo = max(ic_lo, 128 * j0)
                            hi = min(ic_hi, 128 * (j0 + d), T)
                            if lo < hi:
                                regions.append((j0, lo, hi - lo))
                        bins, fills = [], []
                        for j0, lo, w in sorted(regions, key=lambda r: -r[2]):
                            for bi in range(len(bins)):
                                if fills[bi] + w <= 1024:
                                    bins[bi].append((j0, lo, w, fills[bi]))
                                    fills[bi] += w
                                    break
                            else:
                                bins.append([(j0, lo, w, 0)])
                                fills.append(w)
                        pt_map = {}
                        for bin_regions, fill in zip(bins, fills):
                            sps = ps_s.tile([128, 1024], f32, name="sps",
                                            tag="sps")
                            for j0, lo, w, ofs in bin_regions:
                                # matmul output must stay within one PSUM
                                # bank: split pieces at 512 boundaries
                                p0 = ofs
                                while p0 < ofs + w:
                                    p1 = min(ofs + w, (p0 // 512 + 1) * 512)
                                    nc.tensor.matmul(
                                        sps[:, p0:p1],
                                        k_att[s][kr, 128 * j0:128 * (j0 + 1)],
                                        q_att[s][kr, lo + (p0 - ofs):
                                                 lo + (p1 - ofs)],
                                        start=True, stop=True,
                                    )
                                    p0 = p1

                            pt = ptp.tile([128, 1024], bf16, name="pt",
                                          tag="pt")
                            nc.scalar.activation(pt[:, 0:fill], sps[:, 0:fill],
                                                 Exp, bias=zbias[:])
                            for j0, lo, w, ofs in bin_regions:
                                if lo == 128 * j0:
                                    # causal mask on the diagonal block:
                                    # min with {causal side: inf, else: 0}
                                    # zeroes the masked (overflowed) entries;
                                    # bf16 SBUF-only op gets the DVE 4x mode
                                    nc.vector.tensor_tensor(
                                        pt[:, ofs:ofs + 128],
                                        pt[:, ofs:ofs + 128], mcl_s[:],
                                        mybir.AluOpType.min)
                                pt_map[j0] = (pt, lo, ofs)
                        for i0 in range(8 * IC, 8 * (IC + 1)):
                            j_lo = max(0, i0 - d + 1)
                            pcx = ps_c.tile([128, 128], f32, name="pcx", tag="pcx")
                            for j0 in range(j_lo, i0 + 1):
                                pt, lo, ofs = pt_map[j0]
                                off = ofs + 128 * i0 - lo
                                nc.tensor.matmul(
                                    pcx[:, 0:65],
                                    pt[:, off:off + 128],
                                    v4[:, j0, s, :],
                                    start=(j0 == j_lo), stop=(j0 == i0),
                                )
                            rc = small.tile([128, 1], f32, name="rc", tag="rc")
                            nc.vector.reciprocal(rc[:], pcx[:, 64:65])
                            nc.vector.tensor_scalar_mul(
                                ctx_sb[:, i0, 64 * s:64 * (s + 1)],
                                pcx[:, 0:64], rc[:])
                            if fuse is not None:
                                fuse(i0)

                # ---- emission ----
                with ExitStack() as attn_ctx:
                    ps_s = attn_ctx.enter_context(
                        tc.tile_pool(name="ps_s", bufs=2, space="PSUM"))
                    ps_c = attn_ctx.enter_context(
                        tc.tile_pool(name="ps_c", bufs=2, space="PSUM"))
                    for tcq in range(NTC):
                        proj_qk_tc(wq_s, bq_s, q_att, 0, tcq)
                        proj_qk_tc(wk_s, bk_s, k_att, 0, tcq)
                    proj_v()
                    attention(0, ps_s, ps_c)
                    attention(1, ps_s, ps_c)
                    for tcq in range(NTC):
                        proj_qk_tc(wq_s, bq_s, q_att, 1, tcq)
                        proj_qk_tc(wk_s, bk_s, k_att, 1, tcq)
                    attention(2, ps_s, ps_c)

                    # ---- output projection, fused into slot 3 ----
                    # Out-proj block tb only needs ctx_sb[:, tb, :], whose
                    # last writer is slot 3's divide for i0 == tb, so the
                    # matmuls slot in right after it — on hardware they run
                    # under slot 3's (scalar-engine-bound) exp window.
                    # Transposes borrow ps_c slots, matmuls reuse ps_m.
                    def op_transpose(tb):
                        ctts = []
                        for k in range(2):
                            pst = ps_m.tile([128, 128], bf16, name="pst",
                                            tag="psmm")
                            nc.tensor.transpose(
                                pst[:], ctx_sb[:, tb, 128 * k:128 * (k + 1)],
                                id_s[:])
                            ctt = small.tile([128, 128], bf16, name="ctt",
                                             tag="ctt")
                            nc.vector.tensor_copy(ctt[:], pst[:])
                            ctts.append(ctt)
                        return ctts

                    def op_mms(tb, ctts):
                        for oc in range(2):
                            po = ps_m.tile([128, 512], f32, name="po",
                                           tag="psmm")
                            for k in range(2):
                                nc.tensor.matmul(
                                    po[:],
                                    ctts[k][:],
                                    wp_s[:, k, 512 * oc:512 * (oc + 1)],
                                    start=(k == 0), stop=(k == 1),
                                )
                            ob = small.tile([128, 512], f32, name="ob",
                                            tag="ob")
                            if oc == 0:
                                nc.vector.tensor_copy(ob[:], po[:])
                            else:
                                nc.scalar.activation(ob[:], po[:], Copy)
                            dma_eng = nc.sync if oc == 0 else nc.scalar
                            dma_eng.dma_start(
                                out_d[128 * tb:128 * (tb + 1),
                                      512 * oc:512 * (oc + 1)],
                                ob[:])

                    # run the matmuls one block behind the transposes so the
                    # DVE copy of block tb hides under other PE work
                    pending = []

                    def outproj_tb(tb):
                        if pending:
                            op_mms(*pending.pop())
                        pending.append((tb, op_transpose(tb)))

                    if fuse_out:
                        attention(3, ps_s, ps_c, fuse=outproj_tb)
                        op_mms(*pending.pop())
                    else:
                        attention(3, ps_s, ps_c)
                        for tb in range(NB):
                            outproj_tb(tb)
                        op_mms(*pending.pop())


            for _rep in range(nrep):
                emit_once()

    nc.compile()
    return nc


def _prep_core_inputs(core, x, Wq, bq, Wk, bk, Wv):
    b, c = core // HPC, core % HPC
    heads = [HPC * s + c for s in range(HPC)]
    sl = np.concatenate([np.arange(h * HD, (h + 1) * HD) for h in heads])
    slopes = 2.0 ** (-8.0 * (np.asarray(heads, np.float64) + 1) / NH)
    pos = np.arange(T, dtype=np.float32)

    xT = np.ascontiguousarray(x[b].T).astype(BF16)
    wq = np.ascontiguousarray((Wq[sl] / SCALE).T).astype(BF16)
    wk = np.ascontiguousarray((Wk[sl] / SCALE).T).astype(BF16)
    wv = np.ascontiguousarray(Wv[sl].T).astype(BF16)

    def bias_cols(vec):
        # [128, HPC]: even slot s -> rows 0:64, odd slot s -> rows 64:128
        cols = np.zeros((128, HPC), np.float32)
        per_slot = vec.reshape(HPC, HD)
        for s in range(HPC):
            r0 = 0 if s % 2 == 0 else 64
            cols[r0:r0 + 64, s] = per_slot[s]
        return cols

    bq_c = bias_cols((bq[sl] / SCALE).astype(np.float32))
    bk_c = bias_cols(bk[sl].astype(np.float32))
    ones = np.ones(T, np.float32)
    qrows = np.stack([np.stack([ones, (-slopes[s] * pos).astype(np.float32)])
                      for s in range(HPC)]).astype(np.float32)
    krows = np.stack([np.stack([(slopes[s] * pos).astype(np.float32), ones])
                      for s in range(HPC)]).astype(np.float32)
    # post-exp clamp tile for the diagonal block: min(exp, clamp) keeps the
    # causal side (clamp=inf) and zeroes the masked side (clamp=0), where the
    # exp has overflowed to inf
    jj = np.arange(128)
    maskadd = np.where(jj[:, None] <= jj[None, :], np.inf, 0.0).astype(BF16)
    ident = np.eye(128, dtype=BF16)
    return {
        "xT": xT, "wq": wq, "wk": wk, "wv": wv, "bq": bq_c, "bk": bk_c,
        "qrows": qrows, "krows": krows, "maskadd": maskadd, "ident": ident,
    }


def _prep_wp(core, Wp):
    c = core % HPC
    heads = [HPC * s + c for s in range(HPC)]
    sl = np.concatenate([np.arange(h * HD, (h + 1) * HD) for h in heads])
    return np.ascontiguousarray(Wp[:, sl].T).astype(BF16)


def _run(inputs, trace=False):
    from concourse.bass_utils import run_bass_kernel_spmd

    global _PROG
    if _PROG is None:
        _PROG = _build_program()

    x = np.asarray(inputs["x"], np.float32)
    Wq = np.asarray(inputs["Wq"], np.float32)
    bq = np.asarray(inputs["bq"], np.float32)
    Wk = np.asarray(inputs["Wk"], np.float32)
    bk = np.asarray(inputs["bk"], np.float32)
    Wv = np.asarray(inputs["Wv"], np.float32)
    bv = np.asarray(inputs["bv"], np.float32)
    Wp = np.asarray(inputs["Wp"], np.float32)
    bp = np.asarray(inputs["bp"], np.float32)
    assert int(inputs["num_heads"]) == NH

    in_maps = []
    for core in range(8):
        m = _prep_core_inputs(core, x, Wq, bq, Wk, bk, Wv)
        m["wp"] = _prep_wp(core, Wp)
        in_maps.append(m)

    res = run_bass_kernel_spmd(_PROG, in_maps, core_ids=list(range(8)),
                               trace=trace)
    out = np.zeros((B, T, D), np.float32)
    for core in range(8):
        out[core // HPC] += np.asarray(res.results[core]["out"], np.float32)
    out += (bp + bv @ Wp.T)[None, None, :]
    return out, res


def kernel(**inputs) -> np.ndarray:
    out, _ = _run(inputs, trace=False)
    return out

